# revision 52
# baseline (speedup 1.0000x reference)
"""GAT (2-layer, 4-head) Trainium2 kernel, 8-core SPMD.

Strategy:
  - Nodes partitioned across 8 cores by destination (6250 each).
  - Per layer: each core computes the feature table for its node slice
    (feat|el|er packed into 512B fp16 rows with f32 logits via bitcast),
    AllGather -> full table in every core's DRAM.
  - Edges laid out dst-major: each aggregation tile assigns one dst node
    per SBUF partition (degree-balanced permutation); dma_gather pulls
    table[src] rows into slots; attention logits combine gathered el
    (f32) with per-partition er; softmax numerator/denominator reduced
    over slot columns via identity-matmul PSUM accumulation.
  - int16 gather indices: slots split into lo (<32768) and hi column
    blocks gathered from offset table views; er values come from one
    extra gather column against the core-local table slice.
"""

import sys

sys.path.insert(0, "/opt/trn_rl_repo")

import numpy as np

N_CORES = 8
N_NODES = 50000
NPC = N_NODES // N_CORES  # 6250
IN_DIM = 128
HEADS = 4
DIM = 32
HD = HEADS * DIM  # 128
EW = 256          # fp16 elements per table row (512B)
HALF = 32768      # int16 gather index limit
P = 128
TILES = (NPC + P - 1) // P  # 49
GROUP_COLS = 40
CALL_COLS = 7      # <=1024 SWDGE ring descriptors per dma_gather call
NEG_BIG = -1.0e30
EPS = 1e-30


# ----------------------------------------------------------------------------
# host-side slot building (index metadata only)
# ----------------------------------------------------------------------------

def _wrap_idx(idx_flat):
    """[n] -> [128, n/16] int16: i at [i%16 (replicated x8), i//16]."""
    n = idx_flat.shape[0]
    assert n % 16 == 0
    w = idx_flat.reshape(n // 16, 16).T.astype(np.int16)
    return np.tile(w, (8, 1))


def _layer_slots_core(src_id, dst_local):
    """Per-core edge bucketing. Returns (perm, lo_lists, hi_lists) where
    lo_lists[n]/hi_lists[n] are index lists for node-local n."""
    lo_mask = src_id < HALF
    lo_lists = [[] for _ in range(NPC)]
    hi_lists = [[] for _ in range(NPC)]
    for s, d, m in zip(src_id, dst_local, lo_mask):
        (lo_lists if m else hi_lists)[d].append(s if m else s - HALF)
    lo_deg = np.array([len(x) for x in lo_lists])
    hi_deg = np.array([len(x) for x in hi_lists])
    perm = np.lexsort((hi_deg, lo_deg))  # sort nodes by (lo, hi)
    return perm, lo_lists, hi_lists


def _build_layer(cores_src_id, cores_dst_local, er_idx_of_node):
    """Build per-layer slot structure shared across cores.

    cores_src_id/cores_dst_local: per-core arrays.
    er_idx_of_node: list per core of [NPC] arrays giving the table-slice row
      of each node-local id.
    Returns (shared, per_core) where shared has CA/CB/groups and per_core has
    idx (wrapped int16), maskbias, perm.
    """
    percore = []
    for k in range(N_CORES):
        perm, lo_l, hi_l = _layer_slots_core(cores_src_id[k], cores_dst_local[k])
        percore.append((perm, lo_l, hi_l))

    # per-tile max column counts across cores
    CA = np.zeros(TILES, dtype=np.int64)
    CB = np.zeros(TILES, dtype=np.int64)
    for k in range(N_CORES):
        perm, lo_l, hi_l = percore[k]
        for t in range(TILES):
            nodes = perm[t * P: min((t + 1) * P, NPC)]
            ca = max((len(lo_l[n]) for n in nodes), default=0)
            cb = max((len(hi_l[n]) for n in nodes), default=0)
            CA[t] = max(CA[t], ca)
            CB[t] = max(CB[t], cb)
    CA = np.maximum(CA, 1)
    CB = np.maximum(CB, 1)

    # group tiles under a column budget so the G pool stays bounded
    groups = []
    cur = []
    cur_cols = 0
    for t in range(TILES):
        c = int(CA[t] + CB[t] + 1)
        if cur and cur_cols + c > GROUP_COLS:
            groups.append(cur)
            cur = []
            cur_cols = 0
        cur.append(t)
        cur_cols += c
    if cur:
        groups.append(cur)

    out = []
    for k in range(N_CORES):
        perm, lo_l, hi_l = percore[k]
        idx_blocks = []
        mb_cols = []
        for g in groups:
            LO = int(CA[g].sum())
            HI = int(CB[g].sum())
            ER = len(g)
            ilo = np.zeros(LO * P, dtype=np.int64)
            ihi = np.zeros(HI * P, dtype=np.int64)
            ier = np.zeros(ER * P, dtype=np.int64)
            c_lo = 0
            c_hi = 0
            for gi, t in enumerate(g):
                mb_t = np.full((P, int(CA[t] + CB[t])), NEG_BIG, dtype=np.float32)
                for p in range(P):
                    ni = t * P + p
                    if ni >= NPC:
                        continue
                    n = perm[ni]
                    ier[gi * P + p] = er_idx_of_node[k][n]
                    for c, s in enumerate(lo_l[n]):
                        ilo[(c_lo + c) * P + p] = s
                        mb_t[p, c] = 0.0
                    for c, s in enumerate(hi_l[n]):
                        ihi[(c_hi + c) * P + p] = s
                        mb_t[p, CA[t] + c] = 0.0
                c_lo += int(CA[t])
                c_hi += int(CB[t])
                mb_cols.append(mb_t)
            for arr in (ilo, ihi, ier):
                cols = arr.shape[0] // P
                for c0 in range(0, cols, CALL_COLS):
                    c1 = min(c0 + CALL_COLS, cols)
                    idx_blocks.append(_wrap_idx(arr[c0 * P:c1 * P]))
        idx = np.concatenate(idx_blocks, axis=1)
        mb = np.concatenate(mb_cols, axis=1)
        out.append({"idx": idx, "mb": mb, "perm": perm})

    shared = {"CA": CA, "CB": CB, "groups": groups}
    return shared, out


def _blkdiag(al, ar):
    """al/ar [H, D] -> [128, 8] block-diagonal placement (data movement)."""
    blk = np.zeros((HD, 2 * HEADS), dtype=np.float32)
    for h in range(HEADS):
        blk[h * DIM:(h + 1) * DIM, h] = al[h]
        blk[h * DIM:(h + 1) * DIM, HEADS + h] = ar[h]
    return blk


# ----------------------------------------------------------------------------
# device program
# ----------------------------------------------------------------------------

def _build_program(sh1, sh2, IC1, IC2, CE1, CE2):
    import os
    PHASE = int(os.environ.get("GAT_PHASE", "6"))
    SUB = int(os.environ.get("GAT_SUB", "9"))
    import concourse.bass as bass
    import concourse.bacc as bacc
    import concourse.tile as tile
    from concourse import mybir, library_config
    from concourse.masks import make_identity

    f32 = mybir.dt.float32
    f32r = mybir.dt.float32r
    f16 = mybir.dt.float16
    i16 = mybir.dt.int16
    HD4 = HD + 4
    Alu = mybir.AluOpType
    Act = mybir.ActivationFunctionType

    nc = bacc.Bacc("TRN2", target_bir_lowering=False, debug=False,
                   enable_asserts=True, num_devices=N_CORES, num_swdge_queues=4)

    xts = nc.dram_tensor("xts", [P, NPC], f32, kind="ExternalInput")
    W1 = nc.dram_tensor("W1", [IN_DIM, HD], f32, kind="ExternalInput")
    W2 = nc.dram_tensor("W2", [HD, HD], f32, kind="ExternalInput")
    alar1 = nc.dram_tensor("alar1", [HD, 8], f32, kind="ExternalInput")
    alar2 = nc.dram_tensor("alar2", [HD, 8], f32, kind="ExternalInput")
    b1r = nc.dram_tensor("b1r", [P, HD], f32, kind="ExternalInput")
    b2r = nc.dram_tensor("b2r", [P, HD], f32, kind="ExternalInput")
    idx1 = nc.dram_tensor("idx1", [P, IC1], i16, kind="ExternalInput")
    idx2 = nc.dram_tensor("idx2", [P, IC2], i16, kind="ExternalInput")
    mb1 = nc.dram_tensor("mb1", [P, CE1], f32, kind="ExternalInput")
    mb2 = nc.dram_tensor("mb2", [P, CE2], f32, kind="ExternalInput")
    out_d = nc.dram_tensor("out", [NPC, DIM], f32, kind="ExternalOutput")

    with tile.TileContext(nc) as tc:
        with (
            tc.tile_pool(name="const", bufs=1) as cpool,
            tc.tile_pool(name="sb", bufs=2) as sb,
            tc.tile_pool(name="gpool", bufs=3) as gpool,
            tc.tile_pool(name="mpool", bufs=2) as mpool,
            tc.tile_pool(name="stat", bufs=1) as stat,
            tc.tile_pool(name="ps", bufs=2, space="PSUM") as ps,
            tc.tile_pool(name="ponce", bufs=1, space="PSUM") as ponce,
            tc.tile_pool(name="pst", bufs=2, space="PSUM") as pst,
            tc.tile_pool(name="dram", bufs=1, space="DRAM") as dram,
        ):
            nc.gpsimd.load_library(library_config.mlp)

            ident = cpool.tile([P, P], f32)
            make_identity(nc, ident[:])
            ident16 = cpool.tile([P, P], f16)
            make_identity(nc, ident16[:])

            # ---- shared constants
            b1_sb = cpool.tile([P, HD], f32)
            nc.sync.dma_start(b1_sb[:], b1r[:])
            b2_sb = cpool.tile([P, HD], f32)
            nc.sync.dma_start(b2_sb[:], b2r[:])
            b2mean = cpool.tile([P, DIM], f32)
            nc.vector.tensor_reduce(
                out=b2mean[:], in_=b2_sb[:].rearrange("p (h j) -> p j h", h=HEADS),
                op=Alu.add, axis=mybir.AxisListType.X)
            nc.vector.tensor_scalar_mul(b2mean[:], b2mean[:], 0.25)

            def build_aug(Wt, alart):
                W_sb = cpool.tile([P, HD], f32, tag="wtmp")
                nc.sync.dma_start(W_sb[:], Wt[:])
                alar_sb = cpool.tile([P, 8], f32, tag="alartmp")
                nc.sync.dma_start(alar_sb[:], alart[:])
                wt_ps = ponce.tile([P, P], f32, space="PSUM", tag="once")
                nc.tensor.transpose(wt_ps[:], W_sb[:], ident[:])
                wt_sb = cpool.tile([P, P], f32, tag="wT")
                nc.vector.tensor_copy(wt_sb[:], wt_ps[:])
                elr_ps = ponce.tile([P, 8], f32, space="PSUM", tag="once")
                nc.tensor.matmul(out=elr_ps[:], lhsT=wt_sb[:], rhs=alar_sb[:],
                                 start=True, stop=True)
                aug = cpool.tile([P, HD + 8], f32)
                nc.scalar.copy(aug[:, 0:HD], W_sb[:])
                nc.vector.tensor_copy(aug[:, HD:HD + 8], elr_ps[:])
                return aug

            W1aug = build_aug(W1, alar1)
            W2aug = build_aug(W2, alar2)

            # ---- DRAM tables
            t1_slice = dram.tile([NPC, EW], f16)
            t1_full = dram.tile([N_NODES, EW], f16, addr_space="Shared")
            t2_slice = dram.tile([NPC, EW], f16)
            t2_full = dram.tile([N_NODES, EW], f16, addr_space="Shared")

            h_tiles = stat.tile([P, TILES * HD], f32)
            out_sb = stat.tile([P, TILES * DIM], f32)

            # ---- layer-1 table phase
            def table_tile(t, lhs_cols, aug, tslice):
                n0 = t * P
                w = min(n0 + P, NPC) - n0
                tps = pst.tile([P, HD + 8], f32, space="PSUM", tag="tbps")
                nc.tensor.matmul(out=tps[:w, :], lhsT=lhs_cols[:, :w], rhs=aug[:],
                                 start=True, stop=True)
                tb = sb.tile([P, EW], f16, tag="tb")
                nc.scalar.copy(tb[:w, 0:HD], tps[:w, 0:HD])
                nc.vector.tensor_copy(
                    tb[:].bitcast(f32)[:w, 64:72], tps[:w, HD:HD + 8])
                nc.sync.dma_start(tslice[n0:n0 + w, :], tb[:w, :])

            for t in range(TILES):
                n0 = t * P
                w = min(n0 + P, NPC) - n0
                xt_sb = sb.tile([P, P], f32, tag="xt")
                nc.sync.dma_start(xt_sb[:, :w], xts[:, n0:n0 + w])
                table_tile(t, xt_sb[:, :w], W1aug, t1_slice)

            if PHASE >= 2:
                nc.gpsimd.collective_compute(
                    "AllGather", Alu.bypass,
                    replica_groups=[list(range(N_CORES))],
                    ins=[t1_slice[:]], outs=[t1_full[:]])

            # ---- aggregation phase (shared for both layers)
            def agg_layer(shared, idx_t, mb_t, IC, CE, tslice, tfull, epilogue):
                CA, CB, groups = shared["CA"], shared["CB"], shared["groups"]
                idx_sb = stat.tile([P, IC], i16, tag="idx")
                nc.sync.dma_start(idx_sb[:], idx_t[:])
                mb_sb = stat.tile([P, CE], f32, tag="mb")
                nc.sync.dma_start(mb_sb[:], mb_t[:])

                io = 0   # idx column offset (int16 cols)
                eo = 0   # maskbias / e-col offset
                ti = 0   # global tile index
                for g in groups:
                    LO = int(CA[g].sum())
                    HI = int(CB[g].sum())
                    ER = len(g)
                    ncols = LO + HI + ER
                    G = gpool.tile([P, ncols, EW], f16, tag="G")
                    q = 0
                    for blk, view, cnt in (
                        (0, tfull[:], LO),
                        (LO, tfull[HALF:, :], HI),
                        (LO + HI, tslice[:], ER),
                    ):
                        for c0 in range(0, cnt, CALL_COLS):
                            c1 = min(c0 + CALL_COLS, cnt)
                            n = (c1 - c0) * P
                            nc.gpsimd.dma_gather(
                                G[:, blk + c0:blk + c1, :], view,
                                idx_sb[:, io:io + n // 16], n, n, EW,
                                queue_num=q % 4)
                            io += n // 16
                            q += 1
                    Gf = G[:].bitcast(f32)  # [P, ncols, 128] f32 view

                    lo0 = 0
                    hi0 = LO
                    for gi, t in enumerate(g) if SUB >= 2 else []:
                        ca, cb = int(CA[t]), int(CB[t])
                        cc = ca + cb
                        # --- attention logits
                        er = Gf[:, LO + HI + gi, 68:72]  # [P, 4] f32
                        e_t = sb.tile([P, cc * HEADS], f32, tag="e")
                        e3 = e_t[:].rearrange("p (c h) -> p c h", h=HEADS)
                        for (o0, n0, c0) in ((0, ca, lo0), (ca, cb, hi0)):
                            if n0 == 0:
                                continue
                            nc.vector.tensor_tensor(
                                out=e3[:, o0:o0 + n0],
                                in0=Gf[:, c0:c0 + n0, 64:68],
                                in1=er.unsqueeze(1).to_broadcast([P, n0, HEADS]),
                                op=Alu.add)
                        # leaky_relu + mask bias
                        nc.vector.scalar_tensor_tensor(
                            out=e_t[:], in0=e_t[:], scalar=0.2, in1=e_t[:],
                            op0=Alu.mult, op1=Alu.max)
                        nc.vector.tensor_tensor(
                            out=e3[:],
                            in0=e3[:],
                            in1=mb_sb[:, eo:eo + cc].unsqueeze(2)
                                .to_broadcast([P, cc, HEADS]),
                            op=Alu.add)
                        ex_t = sb.tile([P, cc * HEADS], f16, tag="ex")
                        nc.scalar.activation(ex_t[:], e_t[:], Act.Exp)
                        ex3 = ex_t[:].rearrange("p (c h) -> p c h", h=HEADS)
                        # --- scaled messages, ex appended for fused denom
                        M = mpool.tile([P, cc * HD4], f16, tag="M")
                        M4 = M[:].rearrange("p (c x) -> p c x", x=HD4)
                        for (o0, n0, c0) in ((0, ca, lo0), (ca, cb, hi0)):
                            if n0 == 0:
                                continue
                            nc.vector.tensor_tensor(
                                out=M4[:, o0:o0 + n0, 0:HD]
                                    .rearrange("p c (h j) -> p c h j", j=DIM),
                                in0=G[:, c0:c0 + n0, 0:HD]
                                    .rearrange("p c (h j) -> p c h j", j=DIM),
                                in1=ex3[:, o0:o0 + n0].unsqueeze(3)
                                    .to_broadcast([P, n0, HEADS, DIM]),
                                op=Alu.mult)
                        nc.scalar.copy(M4[:, :, HD:HD4], ex3[:])
                        # --- vector pre-sum halves the PE chain length
                        hv = cc // 2
                        nc.vector.tensor_tensor(
                            out=M[:, 0:hv * HD4],
                            in0=M[:, 0:hv * HD4],
                            in1=M[:, (cc - hv) * HD4:cc * HD4],
                            op=Alu.add)
                        wv = cc - hv
                        # --- identity-matmul reduce over slot columns
                        if SUB >= 3:
                            num_ps = ps.tile([P, HD4], f32, space="PSUM", tag="num")
                            for c in range(wv):
                                nc.tensor.matmul(
                                    out=num_ps[:], lhsT=ident16[:],
                                    rhs=M[:, c * HD4:(c + 1) * HD4],
                                    start=(c == 0), stop=(c == wv - 1))
                            if SUB >= 4:
                                epilogue(ti, num_ps)
                        lo0 += ca
                        hi0 += cb
                        eo += cc
                        ti += 1

            # ---- layer-1 epilogue: h = elu(num/den + b1)
            def epi1(t, num_ps):
                dent = sb.tile([P, HEADS], f32, tag="dent")
                nc.vector.tensor_scalar_add(dent[:], num_ps[:, HD:HD4], EPS)
                rcp = sb.tile([P, HEADS], f32, tag="rcp")
                nc.vector.reciprocal(rcp[:], dent[:])
                h0 = sb.tile([P, HD], f32, tag="h0")
                nc.vector.tensor_tensor(
                    out=h0[:].rearrange("p (h j) -> p h j", j=DIM),
                    in0=num_ps[:, 0:HD].rearrange("p (h j) -> p h j", j=DIM),
                    in1=rcp[:].unsqueeze(2).to_broadcast([P, HEADS, DIM]),
                    op=Alu.mult)
                nc.vector.tensor_tensor(out=h0[:], in0=h0[:], in1=b1_sb[:],
                                        op=Alu.add)
                ext = sb.tile([P, HD], f32, tag="hexp")
                nc.scalar.activation(ext[:], h0[:], Act.Exp)
                u = sb.tile([P, HD], f32, tag="hu")
                nc.vector.tensor_scalar(
                    out=u[:], in0=ext[:], scalar1=1.0, scalar2=0.0,
                    op0=Alu.subtract, op1=Alu.min)
                nc.vector.scalar_tensor_tensor(
                    out=h_tiles[:, t * HD:(t + 1) * HD], in0=h0[:], scalar=0.0,
                    in1=u[:], op0=Alu.max, op1=Alu.add)

            if PHASE >= 3:
                agg_layer(sh1, idx1, mb1, IC1, CE1, t1_slice, t1_full, epi1)

            # ---- layer-2 table phase (from h tiles)
            for t in range(TILES) if PHASE >= 4 else []:
                n0 = t * P
                w = min(n0 + P, NPC) - n0
                hT_ps = pst.tile([P, P], f32, space="PSUM", tag="hT")
                nc.tensor.transpose(
                    hT_ps[:], h_tiles[:, t * HD:(t + 1) * HD], ident[:])
                hT_sb = sb.tile([P, P], f32, tag="hTs")
                nc.vector.tensor_copy(hT_sb[:], hT_ps[:])
                table_tile(t, hT_sb[:, :P], W2aug, t2_slice)

            if PHASE >= 5:
                nc.gpsimd.collective_compute(
                    "AllGather", Alu.bypass,
                    replica_groups=[list(range(N_CORES))],
                    ins=[t2_slice[:]], outs=[t2_full[:]])

            # ---- layer-2 epilogue: out = mean_h(num/den) + mean(b2)
            def epi2(t, num_ps):
                dent = sb.tile([P, HEADS], f32, tag="dent")
                nc.vector.tensor_scalar(
                    out=dent[:], in0=num_ps[:, HD:HD4], scalar1=4.0, scalar2=EPS,
                    op0=Alu.mult, op1=Alu.add)
                rcp = sb.tile([P, HEADS], f32, tag="rcp")
                nc.vector.reciprocal(rcp[:], dent[:])
                m0 = sb.tile([P, HD], f32, tag="h0")
                nc.vector.tensor_tensor(
                    out=m0[:].rearrange("p (h j) -> p h j", j=DIM),
                    in0=num_ps[:, 0:HD].rearrange("p (h j) -> p h j", j=DIM),
                    in1=rcp[:].unsqueeze(2).to_broadcast([P, HEADS, DIM]),
                    op=Alu.mult)
                red = sb.tile([P, DIM], f32, tag="red")
                nc.vector.tensor_reduce(
                    out=red[:], in_=m0[:].rearrange("p (h j) -> p j h", h=HEADS),
                    op=Alu.add, axis=mybir.AxisListType.X)
                nc.vector.tensor_tensor(
                    out=out_sb[:, t * DIM:(t + 1) * DIM], in0=red[:],
                    in1=b2mean[:], op=Alu.add)

            if PHASE >= 6:
                agg_layer(sh2, idx2, mb2, IC2, CE2, t2_slice, t2_full, epi2)

            # ---- write output (tile-slot order; host unpermutes)
            if PHASE >= 6:
                for t in range(TILES):
                    n0 = t * P
                    w = min(n0 + P, NPC) - n0
                    nc.sync.dma_start(
                        out_d[n0:n0 + w, :],
                        out_sb[:w, t * DIM:(t + 1) * DIM])

    nc.compile()
    return nc


# ----------------------------------------------------------------------------
# entry point
# ----------------------------------------------------------------------------

_CACHE = {}
_DEBUG = None


def kernel(inputs, src, dst, W1, al1, ar1, b1, W2, al2, ar2, b2):
    from concourse import bass_utils

    x = np.asarray(inputs, dtype=np.float32)
    src = np.asarray(src).astype(np.int64)
    dst = np.asarray(dst).astype(np.int64)
    W1 = np.asarray(W1, dtype=np.float32)
    W2 = np.asarray(W2, dtype=np.float32)
    al1 = np.asarray(al1, dtype=np.float32)
    ar1 = np.asarray(ar1, dtype=np.float32)
    al2 = np.asarray(al2, dtype=np.float32)
    ar2 = np.asarray(ar2, dtype=np.float32)
    b1 = np.asarray(b1, dtype=np.float32)
    b2 = np.asarray(b2, dtype=np.float32)

    # --- per-core edge bucketing by dst
    core_of = dst // NPC
    dst_local = dst % NPC
    src1 = [src[core_of == k] for k in range(N_CORES)]
    dstl = [dst_local[core_of == k] for k in range(N_CORES)]

    # layer 1: table rows natural; er rows = node_local
    er1 = [np.arange(NPC, dtype=np.int64) for _ in range(N_CORES)]
    sh1, pc1 = _build_layer(src1, dstl, er1)

    # layer 2: table2 row of node (c, n) = c*NPC + invperm1_c[n]
    invperm1 = []
    for k in range(N_CORES):
        ip = np.empty(NPC, dtype=np.int64)
        ip[pc1[k]["perm"]] = np.arange(NPC)
        invperm1.append(ip)
    src_core = src // NPC
    src_loc = src % NPC
    src2_global = np.empty_like(src)
    for k in range(N_CORES):
        m = src_core == k
        src2_global[m] = k * NPC + invperm1[k][src_loc[m]]
    src2 = [src2_global[core_of == k] for k in range(N_CORES)]
    er2 = invperm1
    sh2, pc2 = _build_layer(src2, dstl, er2)

    IC1 = pc1[0]["idx"].shape[1]
    IC2 = pc2[0]["idx"].shape[1]
    CE1 = pc1[0]["mb"].shape[1]
    CE2 = pc2[0]["mb"].shape[1]

    import os as _os
    key = (_os.environ.get("GAT_PHASE", "6"), _os.environ.get("GAT_SUB", "9"), IC1, IC2, CE1, CE2,
           tuple(sh1["CA"]), tuple(sh1["CB"]),
           tuple(sh2["CA"]), tuple(sh2["CB"]))
    if key not in _CACHE:
        _CACHE.clear()
        _CACHE[key] = _build_program(sh1, sh2, IC1, IC2, CE1, CE2)
    nc = _CACHE[key]

    xT = np.ascontiguousarray(x.T)
    alar1_b = _blkdiag(al1, ar1)
    alar2_b = _blkdiag(al2, ar2)
    b1_rep = np.tile(b1.reshape(1, HD), (P, 1)).astype(np.float32)
    b2_rep = np.tile(b2.reshape(1, HD), (P, 1)).astype(np.float32)

    in_maps = []
    for k in range(N_CORES):
        in_maps.append({
            "xts": np.ascontiguousarray(xT[:, k * NPC:(k + 1) * NPC]),
            "W1": W1, "W2": W2,
            "alar1": alar1_b, "alar2": alar2_b,
            "b1r": b1_rep, "b2r": b2_rep,
            "idx1": pc1[k]["idx"], "idx2": pc2[k]["idx"],
            "mb1": pc1[k]["mb"], "mb2": pc2[k]["mb"],
        })

    import os as _os2
    _trace = _os2.environ.get("GAT_TRACE") == "1"
    _tkw = {}
    if _trace:
        _tdir = _os2.environ.get("GAT_TRACE_DIR")
        if _tdir:
            _os2.makedirs(_tdir, exist_ok=True)
            _tkw["tmpdir"] = _tdir
        _tkw["trace"] = True
    res = bass_utils.run_bass_kernel_spmd(
        nc, in_maps, core_ids=list(range(N_CORES)), **_tkw)

    global _DEBUG
    _DEBUG = {"res": res, "pc1": pc1, "pc2": pc2, "sh1": sh1, "sh2": sh2}
    out = np.empty((N_NODES, DIM), dtype=np.float32)
    for k in range(N_CORES):
        r = np.asarray(res.results[k]["out"])
        out[k * NPC + pc2[k]["perm"]] = r
    return out

